# revision 7
# baseline (speedup 1.0000x reference)
"""Mistral sliding-window GQA attention + LoRA on 8 trn2 cores — v2.

Sharding: DP2 x TP4. Core c -> batch b=c//4, head-slot s=c%4.
Each core: 8 q heads (2 kv groups of 4), full 2048-token sequence.

v2 layout: bf16 weights + activations (psum accumulates fp32), single pass
over hidden_states (resident 4MB bf16 chunk, two psum passes), attention
interleaved per 512-token chunk against an appended K/V cache, per-chunk
AllGather of bf16 attention outputs overlapped with later chunks, and the
out-projection of chunks 0-2 overlapping the final gather. Scores/q/k stay
fp32r for precision; exp/softmax denominator via ones-matmul as before.
"""
import math
from contextlib import ExitStack

import numpy as np
import ml_dtypes

import concourse.bass as bass
import concourse.mybir as mybir
import concourse.tile as tile
from concourse import bacc
from concourse.bass_utils import run_bass_kernel_spmd
from concourse.masks import make_identity

F32 = mybir.dt.float32
F32R = mybir.dt.float32r
BF16 = mybir.dt.bfloat16
F8 = mybir.dt.float8e4
AF = mybir.ActivationFunctionType
BFDT = ml_dtypes.bfloat16

HID = 4096
S = 2048
D = 128
WIN = 1024
NHQ = 8          # q heads per core
G = 2            # kv groups per core
HG = 4           # q heads per kv group
T = 512          # token chunk
NT = S // T      # 4
NHC = HID // 128  # 32 hidden chunks
NKT = S // 128    # 16 k tiles
LORA_R = 16
SCALE = 1.0 / math.sqrt(D)
LORA_SCALING = 2.0
EDGE_D0 = [-384, -256, -128, 0, 640, 768, 896, 1024]
EDGE_IDX = {d0: i for i, d0 in enumerate(EDGE_D0)}


def ktiles_for(q0):
    return [k0 for k0 in range(0, S, 128) if -(T - 128) <= q0 - k0 <= WIN]


_CACHE = {}


def build_nc(null=False, iters=1, upto="full"):
    key = ("null" if null else "full", iters, upto)
    if key in _CACHE:
        return _CACHE[key]
    nc = bacc.Bacc("TRN2", target_bir_lowering=False, debug=False,
                   num_devices=8)
    d = {}
    for name, shape, dt in [
        ("hst", [HID, S], BF16), ("wq", [HID, 1024], BF16),
        ("wk", [HID, 256], BF16), ("wv", [HID, 256], BF16),
        ("wo", [HID, 1024], BF16), ("aq", [HID, LORA_R], BF16),
        ("bq", [LORA_R, 1024], BF16), ("av", [HID, LORA_R], BF16),
        ("bv", [LORA_R, 256], BF16), ("cost", [128, S], F32),
        ("sint", [128, S], F32),
    ]:
        d[name] = nc.dram_tensor(name, shape, dt, kind="ExternalInput").ap()
    out = nc.dram_tensor("out", [1024, S], F32, kind="ExternalOutput").ap()

    if null:
        _build_null(nc, d, out)
    else:
        _build_body(nc, d, out, iters, upto)
    nc.compile()
    _CACHE[key] = nc
    return nc


def _build_null(nc, d, out):
    with tile.TileContext(nc) as tc:
        with tc.tile_pool(name="sb", bufs=2) as sb:
            t = sb.tile([64, S], F32)
            nc.sync.dma_start(t[:], d["cost"][0:64, :])
            for i in range(8):
                nc.sync.dma_start(out[128 * i:128 * i + 64, :], t[:])


def _build_body(nc, d, out, iters=1, upto="full"):
    with tile.TileContext(nc) as tc, ExitStack() as octx:
        cp = octx.enter_context(tc.tile_pool(name="const", bufs=1))
        dp = octx.enter_context(tc.tile_pool(name="dram", bufs=1, space="DRAM"))

        ident = cp.tile([128, 128], BF16)
        make_identity(nc, ident[:])
        ones_bf = cp.tile([128, 1], BF16)
        nc.gpsimd.memset(ones_bf[:], 1.0)

        spill = dp.tile([NT, NHQ, 128, T], BF16)
        ag = dp.tile([NT, 4 * NHQ, 128, T], BF16)

        for rep in range(iters):
            _one_rep(nc, tc, d, out, rep, ident, ones_bf, spill, ag, upto)


def _one_rep(nc, tc, d, out, rep, ident, ones_bf, spill, ag, upto="full"):
    # LIFO pool stack: sp (whole rep) < pa (until out-proj) < wp (until chunk 3)
    spctx = ExitStack()
    sp = spctx.enter_context(tc.tile_pool(name=f"sp{rep}", bufs=1))
    pactx = ExitStack()
    pa = pactx.enter_context(tc.tile_pool(name=f"pa{rep}", bufs=1))
    wctx = ExitStack()
    wp = wctx.enter_context(tc.tile_pool(name=f"w{rep}", bufs=1))
    opctx = ExitStack()

    # hst chunk 0 first so the first matmuls can start early; then weights.
    hck = wp.tile([128, NHC, T], BF16, tag="hck")
    hst_p = d["hst"].rearrange("(c p) s -> p c s", p=128)
    nc.sync.dma_start(hck[:, :, :], hst_p[:, :, 0:T])

    wq_r = wp.tile([128, NHC, 1024], BF16)
    wq_p = d["wq"].rearrange("(c p) n -> p c n", p=128)
    wk_r = wp.tile([128, NHC, 256], BF16)
    wk_p = d["wk"].rearrange("(c p) n -> p c n", p=128)
    wv_r = wp.tile([128, NHC, 256], BF16)
    wv_p = d["wv"].rearrange("(c p) n -> p c n", p=128)
    for cc in range(0, NHC, 8):
        nc.sync.dma_start(wq_r[:, cc:cc + 8, :], wq_p[:, cc:cc + 8, :])
        nc.sync.dma_start(wk_r[:, cc:cc + 8, :], wk_p[:, cc:cc + 8, :])
        nc.sync.dma_start(wv_r[:, cc:cc + 8, :], wv_p[:, cc:cc + 8, :])
    aq_r = wp.tile([128, NHC, LORA_R], BF16)
    nc.sync.dma_start(aq_r[:], d["aq"].rearrange("(c p) r -> p c r", p=128))
    av_r = wp.tile([128, NHC, LORA_R], BF16)
    nc.sync.dma_start(av_r[:], d["av"].rearrange("(c p) r -> p c r", p=128))
    bq_r = wp.tile([LORA_R, 1024], BF16)
    nc.sync.dma_start(bq_r[:], d["bq"][:, :])
    # rows 32:48 so the lora-v second-stage matmul shares tm's base partition
    bv_r = wp.tile([48, 256], BF16)
    nc.sync.dma_start(bv_r[32:48, :], d["bv"][:, :])

    # persistent per-rep activation state
    qcur = pa.tile([128, NHQ, T], F32R, name="qcur")
    ktg = pa.tile([128, G, S], F32R, name="ktg")
    vng = pa.tile([128, G, NKT, 128], BF16, name="vng")

    def rope_into(ps, cs, sn, dst):
        # dst = ps*cos + rotate_half(ps)*sin, psum f32 in, f32r out.
        # Stage psum -> SBUF on ACT first so the bank frees fast; the DVE
        # rope chain then runs off the critical path.
        st = sp.tile([128, T], F32, tag="rst", bufs=4)
        nc.scalar.copy(st[:], ps[:])
        c1 = sp.tile([128, T], F32, tag="rpc", bufs=1)
        nc.vector.tensor_mul(c1[0:64, :], st[0:64, :], cs[0:64, :])
        nc.vector.tensor_mul(c1[64:128, :], st[64:128, :], cs[64:128, :])
        s1 = sp.tile([128, T], F32, tag="rps", bufs=1)
        nc.vector.tensor_mul(s1[0:64, :], st[64:128, :], sn[64:128, :])
        nc.vector.tensor_mul(s1[64:128, :], st[0:64, :], sn[0:64, :])
        nc.vector.tensor_sub(dst[0:64, :], c1[0:64, :], s1[0:64, :])
        nc.vector.tensor_add(dst[64:128, :], c1[64:128, :], s1[64:128, :])

    for t in range(NT):
        q0 = t * T
        cs = sp.tile([128, T], F32, tag="cs", bufs=1)
        nc.sync.dma_start(cs[:], d["cost"][:, q0:q0 + T])
        sn = sp.tile([128, T], F32, tag="sn", bufs=1)
        nc.sync.dma_start(sn[:], d["sint"][:, q0:q0 + T])

        with tc.tile_pool(name=f"pp{rep}_{t}", bufs=1, space="PSUM") as pp:
            tm = sp.tile([48, T], BF16, tag="tm", bufs=1)
            for pas in range(2):  # pass A: heads 0-3 + kv g0 (+lora), B: 4-7 + g1
                g = pas
                qps = [pp.tile([128, T], F32, tag=f"q{i}", name=f"qps{i}")
                       for i in range(HG)]
                kps = pp.tile([128, T], F32, tag="k")
                vps = pp.tile([128, T], F32, tag="v")
                if pas == 0:
                    lps = pp.tile([48, T], F32, tag="l")
                for hc in range(NHC):
                    h = hck[:, hc, :]
                    for i in range(HG):
                        nc.tensor.matmul(
                            qps[i][:], wq_r[:, hc, 512 * g + 128 * i:
                                            512 * g + 128 * (i + 1)],
                            h, start=(hc == 0), stop=False)
                    nc.tensor.matmul(kps[:], wk_r[:, hc, 128 * g:128 * (g + 1)],
                                     h, start=(hc == 0), stop=(hc == NHC - 1))
                    nc.tensor.matmul(vps[:], wv_r[:, hc, 128 * g:128 * (g + 1)],
                                     h, start=(hc == 0), stop=False)
                    if pas == 0:
                        nc.tensor.matmul(lps[0:16, :], aq_r[:, hc, :], h,
                                         start=(hc == 0), stop=(hc == NHC - 1))
                        nc.tensor.matmul(lps[32:48, :], av_r[:, hc, :], h,
                                         start=(hc == 0), stop=(hc == NHC - 1))
                if pas == 0:
                    nc.vector.tensor_copy(tm[:], lps[:])
                for i in range(HG):
                    hh = g * HG + i
                    nc.tensor.matmul(qps[i][:],
                                     bq_r[:, 128 * hh:128 * (hh + 1)],
                                     tm[0:16, :], start=False, stop=True)
                nc.tensor.matmul(vps[:], bv_r[32:48, 128 * g:128 * (g + 1)],
                                 tm[32:48, :], start=False, stop=True)
                # epilogues: RoPE q/k, transpose v into vng
                for i in range(HG):
                    rope_into(qps[i], cs, sn, qcur[:, g * HG + i, :])
                rope_into(kps, cs, sn, ktg[:, g, q0:q0 + T])
                vev = sp.tile([128, T], BF16, tag="vev", bufs=1)
                nc.vector.tensor_copy(vev[:], vps[:])
                for tt in range(4):
                    vtp = pp.tile([128, 128], BF16, tag="vt")
                    nc.tensor.transpose(
                        vtp[:], vev[:, 128 * tt:128 * (tt + 1)], ident[:])
                    nc.vector.tensor_copy(vng[:, g, 4 * t + tt, :], vtp[:])

        if t < NT - 1:
            # prefetch next hst chunk now: queued ahead of the attention
            # spill DMA, so it starts as soon as pass B's reads finish and
            # lands during chunk t's attention.
            nq0 = q0 + T
            nc.sync.dma_start(hck[:, :, :], hst_p[:, :, nq0:nq0 + T])
        if t == NT - 1:
            wctx.close()   # free wq/wk/wv/hst region for wo + ag readback
            # open the out-proj pool and start its big loads now so they
            # run during chunk 3's attention
            op = opctx.enter_context(tc.tile_pool(name=f"op{rep}", bufs=1))
            wo_r = op.tile([128, NHC, 8, 128], BF16)
            wo_p = d["wo"].rearrange("(c p) n -> p c n", p=128)
            for cc in range(0, NHC, 8):
                dst = wo_r[:, cc:cc + 8, :, :].rearrange("p a b c -> p a (b c)")
                nc.sync.dma_start(dst, wo_p[:, cc:cc + 8, :])
            if upto == "full":
                agb0 = op.tile([128, 4 * NHQ, T], BF16, tag="agb", bufs=2)
                nc.sync.dma_start(agb0[:], ag[0].rearrange("h p s -> p h s"))

        if upto == "proj":
            continue

        # ---------------- attention for chunk t ----------------
        # Software-pipelined emission (depth 3): the AV/denominator matmuls
        # for tile k flush after the scores matmuls of tiles k+1..k+3, so
        # the exp+mask latency never head-of-line-blocks the PE queue.
        with tc.tile_pool(name=f"ap{rep}_{t}", bufs=1, space="PSUM") as ap:
            spl = sp.tile([128, NHQ, T], BF16, tag="spl", bufs=1)
            kts = ktiles_for(q0)
            flat = [(h, ki, k0) for h in range(NHQ)
                    for ki, k0 in enumerate(kts)]
            avps, dnps = {}, {}
            pend = []

            def finish_head(h):
                rc = sp.tile([1, T], F32, tag="rc", bufs=1)
                nc.vector.reciprocal(rc[:], dnps[h][:])
                bc = sp.tile([128, T], F32, tag="bc", bufs=1)
                nc.gpsimd.partition_broadcast(bc[:], rc[:])
                nc.vector.tensor_mul(spl[:, h, :], avps[h][:], bc[:])

            def flush_one():
                h, k0, first, last, at = pend.pop(0)
                nc.tensor.matmul(avps[h][:], vng[:, h // HG, k0 // 128, :],
                                 at[:], start=first, stop=last)
                nc.tensor.matmul(dnps[h][:], ones_bf[:], at[:],
                                 start=first, stop=last)
                if last:
                    finish_head(h)

            for h, ki, k0 in flat:
                first, last = ki == 0, ki == len(kts) - 1
                if first:
                    avps[h] = ap.tile([128, T], F32, tag="avp", bufs=2,
                                      name=f"avp{h}")
                    dnps[h] = ap.tile([1, T], F32, tag="dnp", bufs=2,
                                      name=f"dnp{h}")
                sps = ap.tile([128, T], F32, tag="sps", bufs=4)
                nc.tensor.matmul(sps[:], ktg[:, h // HG, k0:k0 + 128],
                                 qcur[:, h, :], start=True, stop=True)
                at = sp.tile([128, T], BF16, tag="at", bufs=5)
                nc.scalar.activation(at[:], sps[:], AF.Exp)
                d0 = q0 - k0
                if d0 - 127 < 0:
                    # zero where (qq - kk + d0) < 0  (causal)
                    nc.gpsimd.affine_select(
                        out=at[:], in_=at[:], pattern=[[1, T]],
                        compare_op=mybir.AluOpType.is_ge,
                        fill=0.0, base=d0, channel_multiplier=-1)
                if d0 + T - 1 > WIN - 1:
                    # zero where (qq - kk + d0) > WIN-1 (window)
                    nc.gpsimd.affine_select(
                        out=at[:], in_=at[:], pattern=[[-1, T]],
                        compare_op=mybir.AluOpType.is_ge,
                        fill=0.0, base=WIN - 1 - d0, channel_multiplier=1)
                pend.append((h, k0, first, last, at))
                if len(pend) > 4:
                    flush_one()
            while pend:
                flush_one()
            nc.scalar.dma_start(spill[t].rearrange("h p s -> p h s"), spl[:])
            if upto == "full":
                nc.gpsimd.collective_compute(
                    "AllGather", mybir.AluOpType.bypass,
                    replica_groups=[[0, 1, 2, 3], [4, 5, 6, 7]],
                    ins=[spill[t].opt()], outs=[ag[t].opt()])

    if upto != "full":
        opctx.close()
        pactx.close()
        spctx.close()
        return

    # ---------------- output projection ----------------
    with tc.tile_pool(name=f"ops{rep}", bufs=1, space="PSUM") as opp:
        for t in range(NT):
            q0 = t * T
            if t == 0:
                agb = agb0
            else:
                agb = op.tile([128, 4 * NHQ, T], BF16, tag="agb", bufs=2)
                nc.sync.dma_start(agb[:], ag[t].rearrange("h p s -> p h s"))
            psums = [opp.tile([128, T], F32, tag=f"o{oc}", name=f"ops{oc}")
                     for oc in range(8)]
            for H in range(4 * NHQ):
                for oc in range(8):
                    nc.tensor.matmul(psums[oc][:], wo_r[:, H, oc, :],
                                     agb[:, H, :], start=(H == 0),
                                     stop=(H == 4 * NHQ - 1))
            for oc in range(8):
                ev = op.tile([128, T], F32, tag="ev", bufs=2, name=f"ev{oc}")
                nc.scalar.copy(ev[:], psums[oc][:])
                nc.sync.dma_start(out[128 * oc:128 * (oc + 1), q0:q0 + T],
                                  ev[:])
    opctx.close()
    pactx.close()
    spctx.close()


def prep_inputs(inputs):
    hs = np.asarray(inputs["hidden_states"], dtype=np.float32)
    pos = np.asarray(inputs["position_ids"]).astype(np.float64)
    Wq = np.asarray(inputs["Wq"], dtype=np.float32)
    Wk = np.asarray(inputs["Wk"], dtype=np.float32)
    Wv = np.asarray(inputs["Wv"], dtype=np.float32)
    Wo = np.asarray(inputs["Wo"], dtype=np.float32)
    aq = np.asarray(inputs["lora_A_q"], dtype=np.float32)
    bq = np.asarray(inputs["lora_B_q"], dtype=np.float32)
    av = np.asarray(inputs["lora_A_v"], dtype=np.float32)
    bv = np.asarray(inputs["lora_B_v"], dtype=np.float32)

    wq_eff = (Wq * SCALE).astype(BFDT)
    bq_eff = (bq * (LORA_SCALING * SCALE)).astype(BFDT)
    bv_eff = (bv * LORA_SCALING).astype(BFDT)
    wk_b = Wk.astype(BFDT)
    wv_b = Wv.astype(BFDT)
    wo_b = Wo.astype(BFDT)
    aq_b = aq.astype(BFDT)
    av_b = av.astype(BFDT)

    inv_freq = 1.0 / (10000.0 ** (np.arange(0, D, 2, dtype=np.float64) / D))
    tabs = []
    for b in range(2):
        freqs = np.outer(pos[b], inv_freq)          # [S, 64]
        ct = np.cos(freqs).T.astype(np.float32)
        st = np.sin(freqs).T.astype(np.float32)
        # replicated to 128 rows so each rope half reads an aligned copy
        tabs.append((np.ascontiguousarray(np.concatenate([ct, ct], axis=0)),
                     np.ascontiguousarray(np.concatenate([st, st], axis=0))))
    hsT = [np.ascontiguousarray(hs[b].T).astype(BFDT) for b in range(2)]

    in_maps = []
    for c in range(8):
        b, s = divmod(c, 4)
        cos_b, sin_b = tabs[b]
        in_maps.append({
            "hst": hsT[b],
            "wq": np.ascontiguousarray(wq_eff[:, 1024 * s:1024 * (s + 1)]),
            "wk": np.ascontiguousarray(wk_b[:, 256 * s:256 * (s + 1)]),
            "wv": np.ascontiguousarray(wv_b[:, 256 * s:256 * (s + 1)]),
            "wo": np.ascontiguousarray(wo_b[:, 1024 * s:1024 * (s + 1)]),
            "aq": aq_b, "av": av_b,
            "bq": np.ascontiguousarray(bq_eff[:, 1024 * s:1024 * (s + 1)]),
            "bv": np.ascontiguousarray(bv_eff[:, 256 * s:256 * (s + 1)]),
            "cost": cos_b, "sint": sin_b,
        })
    return in_maps


def assemble(results):
    out = np.empty((2, S, HID), dtype=np.float32)
    for c in range(8):
        b, r = divmod(c, 4)
        out[b, :, 1024 * r:1024 * (r + 1)] = results[c]["out"].T
    return out


def run_prepped(in_maps, null=False, iters=1):
    nc = build_nc(null=null, iters=iters)
    return run_bass_kernel_spmd(nc, in_maps, list(range(8)), trace=False)


def kernel(**inputs) -> np.ndarray:
    in_maps = prep_inputs(inputs)
    res = run_prepped(in_maps)
    return assemble(res.results)


# revision 8
# speedup vs baseline: 1.1020x; 1.1020x over previous
"""Mistral sliding-window GQA attention + LoRA on 8 trn2 cores — v2.

Sharding: DP2 x TP4. Core c -> batch b=c//4, head-slot s=c%4.
Each core: 8 q heads (2 kv groups of 4), full 2048-token sequence.

v2 layout: bf16 weights + activations (psum accumulates fp32), single pass
over hidden_states (resident 4MB bf16 chunk, two psum passes), attention
interleaved per 512-token chunk against an appended K/V cache, per-chunk
AllGather of bf16 attention outputs overlapped with later chunks, and the
out-projection of chunks 0-2 overlapping the final gather. Scores/q/k stay
fp32r for precision; exp/softmax denominator via ones-matmul as before.
"""
import math
from contextlib import ExitStack

import numpy as np
import ml_dtypes

import concourse.bass as bass
import concourse.mybir as mybir
import concourse.tile as tile
from concourse import bacc
from concourse.bass_utils import run_bass_kernel_spmd
from concourse.masks import make_identity

F32 = mybir.dt.float32
F32R = mybir.dt.float32r
BF16 = mybir.dt.bfloat16
F8 = mybir.dt.float8e4
AF = mybir.ActivationFunctionType
BFDT = ml_dtypes.bfloat16

HID = 4096
S = 2048
D = 128
WIN = 1024
NHQ = 8          # q heads per core
G = 2            # kv groups per core
HG = 4           # q heads per kv group
T = 512          # token chunk
NT = S // T      # 4
NHC = HID // 128  # 32 hidden chunks
NKT = S // 128    # 16 k tiles
LORA_R = 16
SCALE = 1.0 / math.sqrt(D)
LORA_SCALING = 2.0
EDGE_D0 = [-384, -256, -128, 0, 640, 768, 896, 1024]
EDGE_IDX = {d0: i for i, d0 in enumerate(EDGE_D0)}


def ktiles_for(q0):
    return [k0 for k0 in range(0, S, 128) if -(T - 128) <= q0 - k0 <= WIN]


_CACHE = {}


def build_nc(null=False, iters=1, upto="full"):
    key = ("null" if null else "full", iters, upto)
    if key in _CACHE:
        return _CACHE[key]
    nc = bacc.Bacc("TRN2", target_bir_lowering=False, debug=False,
                   num_devices=8)
    d = {}
    for name, shape, dt in [
        ("hst", [HID, S], BF16), ("wq", [HID, 1024], BF16),
        ("wk", [HID, 256], BF16), ("wv", [HID, 256], BF16),
        ("wo", [HID, 1024], BF16), ("aq", [HID, LORA_R], BF16),
        ("bq", [LORA_R, 1024], BF16), ("av", [HID, LORA_R], BF16),
        ("bv", [LORA_R, 256], BF16), ("cost", [128, S], F32),
        ("sint", [128, S], F32),
    ]:
        d[name] = nc.dram_tensor(name, shape, dt, kind="ExternalInput").ap()
    out = nc.dram_tensor("out", [1024, S], F32, kind="ExternalOutput").ap()

    if null:
        _build_null(nc, d, out)
    else:
        _build_body(nc, d, out, iters, upto)
    nc.compile()
    _CACHE[key] = nc
    return nc


def _build_null(nc, d, out):
    with tile.TileContext(nc) as tc:
        with tc.tile_pool(name="sb", bufs=2) as sb:
            t = sb.tile([64, S], F32)
            nc.sync.dma_start(t[:], d["cost"][0:64, :])
            for i in range(8):
                nc.sync.dma_start(out[128 * i:128 * i + 64, :], t[:])


def _build_body(nc, d, out, iters=1, upto="full"):
    with tile.TileContext(nc) as tc, ExitStack() as octx:
        cp = octx.enter_context(tc.tile_pool(name="const", bufs=1))
        dp = octx.enter_context(tc.tile_pool(name="dram", bufs=1, space="DRAM"))

        ident = cp.tile([128, 128], BF16)
        make_identity(nc, ident[:])
        ones_bf = cp.tile([128, 1], BF16)
        nc.gpsimd.memset(ones_bf[:], 1.0)

        spill = dp.tile([NT, NHQ, 128, T], BF16)
        ag = dp.tile([NT, 4 * NHQ, 128, T], BF16)

        for rep in range(iters):
            _one_rep(nc, tc, d, out, rep, ident, ones_bf, spill, ag, upto)


def _one_rep(nc, tc, d, out, rep, ident, ones_bf, spill, ag, upto="full"):
    # LIFO pool stack: sp (whole rep) < pa (until out-proj) < wp (until chunk 3)
    spctx = ExitStack()
    sp = spctx.enter_context(tc.tile_pool(name=f"sp{rep}", bufs=1))
    pactx = ExitStack()
    pa = pactx.enter_context(tc.tile_pool(name=f"pa{rep}", bufs=1))
    wctx = ExitStack()
    wp = wctx.enter_context(tc.tile_pool(name=f"w{rep}", bufs=1))
    opctx = ExitStack()

    # hst chunk 0 first so the first matmuls can start early; then weights.
    hck = wp.tile([128, NHC, T], BF16, tag="hck")
    hst_p = d["hst"].rearrange("(c p) s -> p c s", p=128)
    nc.sync.dma_start(hck[:, :, :], hst_p[:, :, 0:T])

    wq_r = wp.tile([128, NHC, 1024], BF16)
    wq_p = d["wq"].rearrange("(c p) n -> p c n", p=128)
    wk_r = wp.tile([128, NHC, 256], BF16)
    wk_p = d["wk"].rearrange("(c p) n -> p c n", p=128)
    wv_r = wp.tile([128, NHC, 256], BF16)
    wv_p = d["wv"].rearrange("(c p) n -> p c n", p=128)
    for cc in range(0, NHC, 8):
        nc.sync.dma_start(wq_r[:, cc:cc + 8, :], wq_p[:, cc:cc + 8, :])
        nc.sync.dma_start(wk_r[:, cc:cc + 8, :], wk_p[:, cc:cc + 8, :])
        nc.sync.dma_start(wv_r[:, cc:cc + 8, :], wv_p[:, cc:cc + 8, :])
    aq_r = wp.tile([128, NHC, LORA_R], BF16)
    nc.sync.dma_start(aq_r[:], d["aq"].rearrange("(c p) r -> p c r", p=128))
    av_r = wp.tile([128, NHC, LORA_R], BF16)
    nc.sync.dma_start(av_r[:], d["av"].rearrange("(c p) r -> p c r", p=128))
    bq_r = wp.tile([LORA_R, 1024], BF16)
    nc.sync.dma_start(bq_r[:], d["bq"][:, :])
    # rows 32:48 so the lora-v second-stage matmul shares tm's base partition
    bv_r = wp.tile([48, 256], BF16)
    nc.sync.dma_start(bv_r[32:48, :], d["bv"][:, :])

    # persistent per-rep activation state
    qcur = pa.tile([128, NHQ, T], F32R, name="qcur")
    ktg = pa.tile([128, G, S], F32R, name="ktg")
    vng = pa.tile([128, G, NKT, 128], BF16, name="vng")

    def rope_into(ps, cs, sn, dst):
        # dst = ps*cos + rotate_half(ps)*sin, psum f32 in, f32r out.
        # Stage psum -> SBUF on ACT first so the bank frees fast; the DVE
        # rope chain then runs off the critical path.
        st = sp.tile([128, T], F32, tag="rst", bufs=3)
        nc.scalar.copy(st[:], ps[:])
        c1 = sp.tile([128, T], F32, tag="rpc", bufs=1)
        nc.vector.tensor_mul(c1[0:64, :], st[0:64, :], cs[0:64, :])
        nc.vector.tensor_mul(c1[64:128, :], st[64:128, :], cs[64:128, :])
        s1 = sp.tile([128, T], F32, tag="rps", bufs=1)
        nc.vector.tensor_mul(s1[0:64, :], st[64:128, :], sn[64:128, :])
        nc.vector.tensor_mul(s1[64:128, :], st[0:64, :], sn[0:64, :])
        nc.vector.tensor_sub(dst[0:64, :], c1[0:64, :], s1[0:64, :])
        nc.vector.tensor_add(dst[64:128, :], c1[64:128, :], s1[64:128, :])

    for t in range(NT):
        q0 = t * T
        cs = sp.tile([128, T], F32, tag="cs", bufs=1)
        nc.sync.dma_start(cs[:], d["cost"][:, q0:q0 + T])
        sn = sp.tile([128, T], F32, tag="sn", bufs=1)
        nc.sync.dma_start(sn[:], d["sint"][:, q0:q0 + T])

        with tc.tile_pool(name=f"pp{rep}_{t}", bufs=1, space="PSUM") as pp:
            tm = sp.tile([48, T], BF16, tag="tm", bufs=1)
            for pas in range(2):  # pass A: heads 0-3 + kv g0 (+lora), B: 4-7 + g1
                g = pas
                qps = [pp.tile([128, T], F32, tag=f"q{i}", name=f"qps{i}")
                       for i in range(HG)]
                kps = pp.tile([128, T], F32, tag="k")
                vps = pp.tile([128, T], F32, tag="v")
                if pas == 0:
                    lps = pp.tile([48, T], F32, tag="l")
                for hc in range(NHC):
                    h = hck[:, hc, :]
                    for i in range(HG):
                        nc.tensor.matmul(
                            qps[i][:], wq_r[:, hc, 512 * g + 128 * i:
                                            512 * g + 128 * (i + 1)],
                            h, start=(hc == 0), stop=False)
                    nc.tensor.matmul(kps[:], wk_r[:, hc, 128 * g:128 * (g + 1)],
                                     h, start=(hc == 0), stop=(hc == NHC - 1))
                    nc.tensor.matmul(vps[:], wv_r[:, hc, 128 * g:128 * (g + 1)],
                                     h, start=(hc == 0), stop=False)
                    if pas == 0:
                        nc.tensor.matmul(lps[0:16, :], aq_r[:, hc, :], h,
                                         start=(hc == 0), stop=(hc == NHC - 1))
                        nc.tensor.matmul(lps[32:48, :], av_r[:, hc, :], h,
                                         start=(hc == 0), stop=(hc == NHC - 1))
                if pas == 0:
                    nc.vector.tensor_copy(tm[:], lps[:])
                for i in range(HG):
                    hh = g * HG + i
                    nc.tensor.matmul(qps[i][:],
                                     bq_r[:, 128 * hh:128 * (hh + 1)],
                                     tm[0:16, :], start=False, stop=True)
                nc.tensor.matmul(vps[:], bv_r[32:48, 128 * g:128 * (g + 1)],
                                 tm[32:48, :], start=False, stop=True)
                # epilogues: RoPE q/k, transpose v into vng
                for i in range(HG):
                    rope_into(qps[i], cs, sn, qcur[:, g * HG + i, :])
                rope_into(kps, cs, sn, ktg[:, g, q0:q0 + T])
                vev = sp.tile([128, T], BF16, tag="vev", bufs=1)
                nc.vector.tensor_copy(vev[:], vps[:])
                for tt in range(4):
                    vtp = pp.tile([128, 128], BF16, tag="vt")
                    nc.tensor.transpose(
                        vtp[:], vev[:, 128 * tt:128 * (tt + 1)], ident[:])
                    nc.vector.tensor_copy(vng[:, g, 4 * t + tt, :], vtp[:])

        if t < NT - 1:
            # prefetch next hst chunk now: queued ahead of the attention
            # spill DMA, so it starts as soon as pass B's reads finish and
            # lands during chunk t's attention.
            nq0 = q0 + T
            nc.sync.dma_start(hck[:, :, :], hst_p[:, :, nq0:nq0 + T])
        if t == NT - 1:
            wctx.close()   # free wq/wk/wv/hst region for wo + ag readback
            # open the out-proj pool and start its big loads now so they
            # run during chunk 3's attention
            op = opctx.enter_context(tc.tile_pool(name=f"op{rep}", bufs=1))
            wo_r = op.tile([128, NHC, 8, 128], BF16)
            wo_p = d["wo"].rearrange("(c p) n -> p c n", p=128)
            for cc in range(0, NHC, 8):
                dst = wo_r[:, cc:cc + 8, :, :].rearrange("p a b c -> p a (b c)")
                nc.sync.dma_start(dst, wo_p[:, cc:cc + 8, :])
            if upto == "full":
                agb0 = op.tile([128, 4 * NHQ, T], BF16, tag="agb", bufs=2)
                nc.sync.dma_start(agb0[:], ag[0].rearrange("h p s -> p h s"))

        if upto == "proj":
            continue

        # ---------------- attention for chunk t ----------------
        # Software-pipelined emission (depth 3): the AV/denominator matmuls
        # for tile k flush after the scores matmuls of tiles k+1..k+3, so
        # the exp+mask latency never head-of-line-blocks the PE queue.
        with tc.tile_pool(name=f"ap{rep}_{t}", bufs=1, space="PSUM") as ap:
            spl = sp.tile([128, NHQ, T], BF16, tag="spl", bufs=1)
            kts = ktiles_for(q0)
            flat = [(h, ki, k0) for h in range(NHQ)
                    for ki, k0 in enumerate(kts)]
            avps, dnps = {}, {}
            pend = []

            def finish_head(h):
                rc = sp.tile([1, T], F32, tag="rc", bufs=2)
                nc.vector.reciprocal(rc[:], dnps[h][:])
                bc = sp.tile([128, T], F32, tag="bc", bufs=1)
                nc.gpsimd.partition_broadcast(bc[:], rc[:])
                nc.vector.tensor_mul(spl[:, h, :], avps[h][:], bc[:])

            def flush_one():
                h, k0, first, last, at = pend.pop(0)
                nc.tensor.matmul(avps[h][:], vng[:, h // HG, k0 // 128, :],
                                 at[:], start=first, stop=last)
                nc.tensor.matmul(dnps[h][:], ones_bf[:], at[:],
                                 start=first, stop=last)
                if last:
                    finish_head(h)

            for h, ki, k0 in flat:
                first, last = ki == 0, ki == len(kts) - 1
                if first:
                    avps[h] = ap.tile([128, T], F32, tag="avp", bufs=2,
                                      name=f"avp{h}")
                    dnps[h] = ap.tile([1, T], F32, tag="dnp", bufs=2,
                                      name=f"dnp{h}")
                sps = ap.tile([128, T], F32, tag="sps", bufs=4)
                nc.tensor.matmul(sps[:], ktg[:, h // HG, k0:k0 + 128],
                                 qcur[:, h, :], start=True, stop=True)
                at = sp.tile([128, T], BF16, tag="at", bufs=5)
                nc.scalar.activation(at[:], sps[:], AF.Exp)
                d0 = q0 - k0
                if d0 - 127 < 0:
                    # zero where (qq - kk + d0) < 0  (causal)
                    nc.gpsimd.affine_select(
                        out=at[:], in_=at[:], pattern=[[1, T]],
                        compare_op=mybir.AluOpType.is_ge,
                        fill=0.0, base=d0, channel_multiplier=-1)
                if d0 + T - 1 > WIN - 1:
                    # zero where (qq - kk + d0) > WIN-1 (window)
                    nc.gpsimd.affine_select(
                        out=at[:], in_=at[:], pattern=[[-1, T]],
                        compare_op=mybir.AluOpType.is_ge,
                        fill=0.0, base=WIN - 1 - d0, channel_multiplier=1)
                pend.append((h, k0, first, last, at))
                if len(pend) > 4:
                    flush_one()
            while pend:
                flush_one()
            nc.scalar.dma_start(spill[t].rearrange("h p s -> p h s"), spl[:])
            if upto == "full":
                nc.gpsimd.collective_compute(
                    "AllGather", mybir.AluOpType.bypass,
                    replica_groups=[[0, 1, 2, 3], [4, 5, 6, 7]],
                    ins=[spill[t].opt()], outs=[ag[t].opt()])

    if upto != "full":
        opctx.close()
        pactx.close()
        spctx.close()
        return

    # ---------------- output projection ----------------
    with tc.tile_pool(name=f"ops{rep}", bufs=1, space="PSUM") as opp:
        for t in range(NT):
            q0 = t * T
            if t == 0:
                agb = agb0
            else:
                agb = op.tile([128, 4 * NHQ, T], BF16, tag="agb", bufs=2)
                nc.sync.dma_start(agb[:], ag[t].rearrange("h p s -> p h s"))
            psums = [opp.tile([128, T], F32, tag=f"o{oc}", name=f"ops{oc}")
                     for oc in range(8)]
            for H in range(4 * NHQ):
                for oc in range(8):
                    nc.tensor.matmul(psums[oc][:], wo_r[:, H, oc, :],
                                     agb[:, H, :], start=(H == 0),
                                     stop=(H == 4 * NHQ - 1))
            for oc in range(8):
                ev = op.tile([128, T], F32, tag="ev", bufs=2, name=f"ev{oc}")
                nc.scalar.copy(ev[:], psums[oc][:])
                nc.sync.dma_start(out[128 * oc:128 * (oc + 1), q0:q0 + T],
                                  ev[:])
    opctx.close()
    pactx.close()
    spctx.close()


def prep_inputs(inputs):
    hs = np.asarray(inputs["hidden_states"], dtype=np.float32)
    pos = np.asarray(inputs["position_ids"]).astype(np.float64)
    Wq = np.asarray(inputs["Wq"], dtype=np.float32)
    Wk = np.asarray(inputs["Wk"], dtype=np.float32)
    Wv = np.asarray(inputs["Wv"], dtype=np.float32)
    Wo = np.asarray(inputs["Wo"], dtype=np.float32)
    aq = np.asarray(inputs["lora_A_q"], dtype=np.float32)
    bq = np.asarray(inputs["lora_B_q"], dtype=np.float32)
    av = np.asarray(inputs["lora_A_v"], dtype=np.float32)
    bv = np.asarray(inputs["lora_B_v"], dtype=np.float32)

    wq_eff = (Wq * SCALE).astype(BFDT)
    bq_eff = (bq * (LORA_SCALING * SCALE)).astype(BFDT)
    bv_eff = (bv * LORA_SCALING).astype(BFDT)
    wk_b = Wk.astype(BFDT)
    wv_b = Wv.astype(BFDT)
    wo_b = Wo.astype(BFDT)
    aq_b = aq.astype(BFDT)
    av_b = av.astype(BFDT)

    inv_freq = 1.0 / (10000.0 ** (np.arange(0, D, 2, dtype=np.float64) / D))
    tabs = []
    for b in range(2):
        freqs = np.outer(pos[b], inv_freq)          # [S, 64]
        ct = np.cos(freqs).T.astype(np.float32)
        st = np.sin(freqs).T.astype(np.float32)
        # replicated to 128 rows so each rope half reads an aligned copy
        tabs.append((np.ascontiguousarray(np.concatenate([ct, ct], axis=0)),
                     np.ascontiguousarray(np.concatenate([st, st], axis=0))))
    hsT = [np.ascontiguousarray(hs[b].T).astype(BFDT) for b in range(2)]

    in_maps = []
    for c in range(8):
        b, s = divmod(c, 4)
        cos_b, sin_b = tabs[b]
        in_maps.append({
            "hst": hsT[b],
            "wq": np.ascontiguousarray(wq_eff[:, 1024 * s:1024 * (s + 1)]),
            "wk": np.ascontiguousarray(wk_b[:, 256 * s:256 * (s + 1)]),
            "wv": np.ascontiguousarray(wv_b[:, 256 * s:256 * (s + 1)]),
            "wo": np.ascontiguousarray(wo_b[:, 1024 * s:1024 * (s + 1)]),
            "aq": aq_b, "av": av_b,
            "bq": np.ascontiguousarray(bq_eff[:, 1024 * s:1024 * (s + 1)]),
            "bv": np.ascontiguousarray(bv_eff[:, 256 * s:256 * (s + 1)]),
            "cost": cos_b, "sint": sin_b,
        })
    return in_maps


def assemble(results):
    out = np.empty((2, S, HID), dtype=np.float32)
    for c in range(8):
        b, r = divmod(c, 4)
        out[b, :, 1024 * r:1024 * (r + 1)] = results[c]["out"].T
    return out


def run_prepped(in_maps, null=False, iters=1):
    nc = build_nc(null=null, iters=iters)
    return run_bass_kernel_spmd(nc, in_maps, list(range(8)), trace=False)


def kernel(**inputs) -> np.ndarray:
    in_maps = prep_inputs(inputs)
    res = run_prepped(in_maps)
    return assemble(res.results)
